# revision 21
# baseline (speedup 1.0000x reference)
"""Gaussian density-grid kernel for Trainium2 (8 NeuronCores).

density[g] = sum_{a,n} aw[a,n]*mask[a] * exp(bw[a,n] * ||grid_g - X_a||^2)

The grid is a regular 48^3 lattice, so the Gaussian factorizes per axis:
    exp(bw*(dx^2+dy^2+dz^2)) = Ex(i) * Ey(j) * Ez(k)
Per (atom, gaussian) pair p we build three 48-entry 1D tables, form the
outer product Ey (x) Ez on the vector engine, and contract over p with the
tensor engine:  out[i, (k,j)] = sum_p (w*Ex)[p,i] * (Ey*Ez)[p,(k,j)].

Active (mask==1) atoms are compacted on the host and the p axis is sharded
across the 8 cores; each core produces a partial density over the full grid
and the host sums the 8 partials.
"""

import math

import numpy as np

NXYZ = 48
G2D = NXYZ * NXYZ  # 2304 (k,j) pairs
G = NXYZ * G2D
N_CORES = 8
P_TILE = 128
NCOEF = 5  # s, -s*x, -s*y, -s*z, log(w) per chunk
ZBLK = 8  # z-rows per M piece -> N = ZBLK*48 = 384 per matmul
NPIECE = NXYZ // ZBLK  # 6 matmul pieces per chunk

# matmul operand dtype: "f32r" (1 cyc/row, reduced-precision multiply),
# "f32" (4 cyc/row, exact) or "bf16"
import os as _os

MM_DTYPE = _os.environ.get("DENS_MM_DTYPE", "f32r")


def _build_program(n_chunks: int):
    import concourse.mybir as mybir
    import concourse.tile as tile
    from concourse import bacc
    from concourse.alu_op_type import AluOpType

    f32 = mybir.dt.float32
    ACT = mybir.ActivationFunctionType

    nc = bacc.Bacc(
        "TRN2",
        target_bir_lowering=False,
        debug=False,
        enable_asserts=False,
        num_devices=N_CORES,
    )

    # packed input: [coef (NCOEF*n_chunks) | ay | az | ax]
    inp_w = 3 * NXYZ + NCOEF * n_chunks
    wa = NCOEF * n_chunks + 2 * NXYZ  # first DMA: coef + ay + az
    inp_d = nc.dram_tensor("inp", [P_TILE, inp_w], f32, kind="ExternalInput")
    dens_d = nc.dram_tensor("dens", [G], f32, kind="ExternalOutput")

    if MM_DTYPE == "bf16":
        mm_dt = mybir.dt.bfloat16
    elif MM_DTYPE == "f32r":
        mm_dt = mybir.dt.float32r
    else:
        mm_dt = f32

    with tile.TileContext(nc) as tc:
        with (
            tc.tile_pool(name="const", bufs=1) as cpool,
            tc.tile_pool(name="work", bufs=3) as wpool,
            tc.tile_pool(name="mbuf", bufs=2) as mpool,
            tc.tile_pool(name="outs", bufs=1) as opool,
            tc.tile_pool(name="acc", bufs=1, space="PSUM") as acc_pool,
        ):
            # dummy activation with no data deps: forces the ACT table load
            # to issue before the DMA-wait blocks the Scalar queue
            dummy = cpool.tile([P_TILE, 1], f32)
            nc.scalar.activation(dummy[:], dummy[:], ACT.Exp, bias=0.0, scale=0.0)

            # two tiles (not one) so the two DMAs have no false WAW dep
            inp_a = cpool.tile([P_TILE, wa], f32)
            nc.sync.dma_start(inp_a[:], inp_d.ap()[:, 0:wa])
            inp_b = cpool.tile([P_TILE, NXYZ], f32)
            nc.gpsimd.dma_start(inp_b[:], inp_d.ap()[:, wa:inp_w])
            ay_b = inp_a[:, NCOEF * n_chunks : NCOEF * n_chunks + NXYZ]
            az_b = inp_a[:, NCOEF * n_chunks + NXYZ : NCOEF * n_chunks + 2 * NXYZ]
            ax_b = inp_b[:, 0:NXYZ]
            coef_off = 0

            # PSUM accumulators [48, 384] per z-piece, accumulated over chunks
            accs = [
                acc_pool.tile([NXYZ, ZBLK * NXYZ], f32, tag=f"acc{b}", name=f"acc{b}")
                for b in range(NPIECE)
            ]

            # Stage 1: per-chunk 1D Gaussian tables (kept alive for all pieces)
            exs, eyzs = [], []
            for c in range(n_chunks):
                o = coef_off + c * NCOEF
                s_c = inp_a[:, o : o + 1]
                ntx = inp_a[:, o + 1 : o + 2]
                nty = inp_a[:, o + 2 : o + 3]
                ntz = inp_a[:, o + 3 : o + 4]
                lw = inp_a[:, o + 4 : o + 5]

                # SQ[:, axis_block] = (s*coord - s*center)^2 = -bw * d^2
                # yz first so the M outer-product (needs eyz only) starts early
                sq = wpool.tile([P_TILE, 3 * NXYZ], f32, tag="sq", name=f"sq{c}")
                if c == 0:
                    # chunk 0 on the idle-at-start Vector engine (tensor_scalar
                    # runs 2x for fp32) to shorten the ACT prefix
                    u = wpool.tile([P_TILE, 2 * NXYZ], f32, tag="u", name="u0")
                    nc.vector.tensor_scalar(
                        u[:, 0:NXYZ], ay_b, s_c, nty, AluOpType.mult, AluOpType.add
                    )
                    nc.vector.tensor_scalar(
                        u[:, NXYZ : 2 * NXYZ], az_b, s_c, ntz, AluOpType.mult, AluOpType.add
                    )
                    nc.vector.tensor_tensor(
                        sq[:, NXYZ : 3 * NXYZ], u[:], u[:], AluOpType.mult
                    )
                else:
                    for blk, bias, coord in ((1, nty, ay_b), (2, ntz, az_b)):
                        nc.scalar.activation(
                            sq[:, blk * NXYZ : (blk + 1) * NXYZ],
                            coord,
                            ACT.Square,
                            bias=bias,
                            scale=s_c,
                        )
                eyz_dt = mm_dt if MM_DTYPE == "bf16" else f32
                eyz = wpool.tile(
                    [P_TILE, 2 * NXYZ], eyz_dt, tag="eyz", name=f"eyz{c}", bufs=n_chunks
                )
                nc.scalar.activation(
                    eyz[:], sq[:, NXYZ : 3 * NXYZ], ACT.Exp, bias=0.0, scale=-1.0
                )
                # Ex = exp(-SQx + log w)  (weight folded in)
                nc.scalar.activation(
                    sq[:, 0:NXYZ], ax_b, ACT.Square, bias=ntx, scale=s_c
                )
                ex = wpool.tile([P_TILE, NXYZ], mm_dt, tag="ex", name=f"ex{c}", bufs=n_chunks)
                nc.scalar.activation(ex[:], sq[:, 0:NXYZ], ACT.Exp, bias=lw, scale=-1.0)
                exs.append(ex)
                eyzs.append(eyz)

            # Stage 2: c-outer sweeps (no DVE head-of-line stall on chunk-1
            # tables); piece b drains during the final sweep right after its
            # stop matmul, overlapping the remaining pieces' work
            out_s = opool.tile([NXYZ, G2D], f32)
            for c in range(n_chunks):
                for b in range(NPIECE):
                    ey = eyzs[c][:, 0:NXYZ]
                    ez_b = eyzs[c][:, NXYZ + b * ZBLK : NXYZ + (b + 1) * ZBLK]
                    m_t = mpool.tile(
                        [P_TILE, ZBLK * NXYZ], mm_dt, tag="m", name=f"m{b}_{c}", bufs=8
                    )
                    nc.vector.tensor_tensor(
                        m_t[:].rearrange("p (z j) -> p z j", z=ZBLK),
                        ey.unsqueeze(1).broadcast_to((P_TILE, ZBLK, NXYZ)),
                        ez_b.unsqueeze(2).broadcast_to((P_TILE, ZBLK, NXYZ)),
                        AluOpType.mult,
                    )
                    nc.tensor.matmul(
                        accs[b][:],
                        exs[c][:],
                        m_t[:],
                        start=(c == 0),
                        stop=(c == n_chunks - 1),
                    )

                    if c == n_chunks - 1:
                        # drain piece b: psum -> sbuf on Scalar (idle after
                        # tables; keeps Vector on pure outer-product work)
                        dst = out_s[:, b * ZBLK * NXYZ : (b + 1) * ZBLK * NXYZ]
                        nc.scalar.copy(dst, accs[b][:])
                        if b % 2 == 1:
                            # one DMA per piece-pair, rotating queues (i-major;
                            # host transposes to (z,j,i) while summing partials)
                            lo = (b - 1) * ZBLK * NXYZ
                            hi = (b + 1) * ZBLK * NXYZ
                            pair = out_s[:, lo:hi]
                            dens_pc = dens_d.ap().rearrange(
                                "(i zj) -> i zj", i=NXYZ
                            )[:, lo:hi]
                            dma_eng = (nc.sync, nc.gpsimd, nc.scalar)[(b - 1) // 2]
                            with nc.allow_non_contiguous_dma("strided output store"):
                                dma_eng.dma_start(dens_pc, pair)

    nc.compile()
    return nc


def _host_prep(X, aw, bw, elements, C_expand, real_grid):
    ax = np.ascontiguousarray(real_grid[0:NXYZ, 0])
    ay = np.ascontiguousarray(real_grid[0 : NXYZ * NXYZ : NXYZ, 1])
    az = np.ascontiguousarray(real_grid[0 : G : NXYZ * NXYZ, 2])

    mask = (elements != 5) & (C_expand == 1)
    act = np.nonzero(mask)[0]
    # per-(atom, gaussian) flattened arrays over active atoms
    bw_p = bw[act].reshape(-1).astype(np.float64)
    aw_p = aw[act].reshape(-1).astype(np.float64)
    x_p = np.repeat(X[act, 0].astype(np.float64), 6)
    y_p = np.repeat(X[act, 1].astype(np.float64), 6)
    z_p = np.repeat(X[act, 2].astype(np.float64), 6)
    p_act = bw_p.shape[0]

    per_core = max(1, math.ceil(p_act / (N_CORES * P_TILE))) * P_TILE
    n_chunks = per_core // P_TILE

    s_p = np.sqrt(-bw_p)
    coef_full = np.zeros((N_CORES * per_core, NCOEF), dtype=np.float32)
    coef_full[:, 4] = -1e4  # padding rows: exp(-1e4) -> 0
    coef_full[:p_act, 0] = s_p
    coef_full[:p_act, 1] = -s_p * x_p
    coef_full[:p_act, 2] = -s_p * y_p
    coef_full[:p_act, 3] = -s_p * z_p
    coef_full[:p_act, 4] = np.log(aw_p)

    # core/chunk/partition layout: [core][chunk][row(128)] -> [row, chunk*NCOEF+j]
    coefs = []
    for core in range(N_CORES):
        cc = coef_full[core * per_core : (core + 1) * per_core]  # [per_core, NCOEF]
        cc = cc.reshape(n_chunks, P_TILE, NCOEF).transpose(1, 0, 2).reshape(P_TILE, -1)
        coefs.append(np.ascontiguousarray(cc))

    # packed per-core input: [axs (3*48) | coef (NCOEF*n_chunks)]
    inps = []
    for core in range(N_CORES):
        nco = NCOEF * n_chunks
        inp = np.empty((P_TILE, 3 * NXYZ + nco), dtype=np.float32)
        inp[:, 0:nco] = coefs[core]
        inp[:, nco : nco + NXYZ] = ay[None, :]
        inp[:, nco + NXYZ : nco + 2 * NXYZ] = az[None, :]
        inp[:, nco + 2 * NXYZ :] = ax[None, :]
        inps.append(inp)
    return inps, n_chunks


_prog_cache = {}


def kernel(X, aw, bw, elements, C_expand, real_grid, _trace=False):
    from concourse import bass_utils

    inps, n_chunks = _host_prep(X, aw, bw, elements, C_expand, real_grid)

    key = (n_chunks, MM_DTYPE)
    if key not in _prog_cache:
        _prog_cache[key] = _build_program(n_chunks)
    nc = _prog_cache[key]

    in_maps = [{"inp": inps[core]} for core in range(N_CORES)]
    res = bass_utils.run_bass_kernel_spmd(
        nc, in_maps, core_ids=list(range(N_CORES)), trace=_trace
    )
    dens = np.zeros((G2D, NXYZ), dtype=np.float64)
    for core in range(N_CORES):
        dens += res.results[core]["dens"].reshape(NXYZ, G2D).T
    out = np.ascontiguousarray(dens.reshape(-1)).astype(np.float32)
    if _trace:
        return out, res
    return out


# revision 22
# speedup vs baseline: 1.0327x; 1.0327x over previous
"""Gaussian density-grid kernel for Trainium2 (8 NeuronCores).

density[g] = sum_{a,n} aw[a,n]*mask[a] * exp(bw[a,n] * ||grid_g - X_a||^2)

The grid is a regular 48^3 lattice, so the Gaussian factorizes per axis:
    exp(bw*(dx^2+dy^2+dz^2)) = Ex(i) * Ey(j) * Ez(k)
Per (atom, gaussian) pair p we build three 48-entry 1D tables, form the
outer product Ey (x) Ez on the vector engine, and contract over p with the
tensor engine:  out[i, (k,j)] = sum_p (w*Ex)[p,i] * (Ey*Ez)[p,(k,j)].

Active (mask==1) atoms are compacted on the host and the p axis is sharded
across the 8 cores; each core produces a partial density over the full grid
and the host sums the 8 partials.
"""

import math

import numpy as np

NXYZ = 48
G2D = NXYZ * NXYZ  # 2304 (k,j) pairs
G = NXYZ * G2D
N_CORES = 8
P_TILE = 128
NCOEF = 5  # s, -s*x, -s*y, -s*z, log(w) per chunk
ZBLK = 8  # z-rows per M piece -> N = ZBLK*48 = 384 per matmul
NPIECE = NXYZ // ZBLK  # 6 matmul pieces per chunk

# matmul operand dtype: "f32r" (1 cyc/row, reduced-precision multiply),
# "f32" (4 cyc/row, exact) or "bf16"
import os as _os

MM_DTYPE = _os.environ.get("DENS_MM_DTYPE", "f32r")


def _build_program(n_chunks: int):
    import concourse.mybir as mybir
    import concourse.tile as tile
    from concourse import bacc
    from concourse.alu_op_type import AluOpType
    from concourse.tile_rust import add_dep_helper

    f32 = mybir.dt.float32
    ACT = mybir.ActivationFunctionType

    nc = bacc.Bacc(
        "TRN2",
        target_bir_lowering=False,
        debug=False,
        enable_asserts=False,
        num_devices=N_CORES,
    )

    # packed input: [coef (NCOEF*n_chunks) | ay | az | ax]
    inp_w = 3 * NXYZ + NCOEF * n_chunks
    wa = NCOEF * n_chunks + 2 * NXYZ  # first DMA: coef + ay + az
    inp_d = nc.dram_tensor("inp", [P_TILE, inp_w], f32, kind="ExternalInput")
    dens_d = nc.dram_tensor("dens", [G], f32, kind="ExternalOutput")

    if MM_DTYPE == "bf16":
        mm_dt = mybir.dt.bfloat16
    elif MM_DTYPE == "f32r":
        mm_dt = mybir.dt.float32r
    else:
        mm_dt = f32

    with tile.TileContext(nc) as tc:
        with (
            tc.tile_pool(name="const", bufs=1) as cpool,
            tc.tile_pool(name="work", bufs=3) as wpool,
            tc.tile_pool(name="mbuf", bufs=2) as mpool,
            tc.tile_pool(name="outs", bufs=1) as opool,
            tc.tile_pool(name="acc", bufs=1, space="PSUM") as acc_pool,
        ):
            # dummy activation with no data deps: forces the ACT table load
            # to issue before the DMA-wait blocks the Scalar queue
            dummy = cpool.tile([P_TILE, 1], f32)
            nc.scalar.activation(dummy[:], dummy[:], ACT.Exp, bias=0.0, scale=0.0)

            # two tiles (not one) so the two DMAs have no false WAW dep
            inp_a = cpool.tile([P_TILE, wa], f32)
            nc.sync.dma_start(inp_a[:], inp_d.ap()[:, 0:wa])
            inp_b = cpool.tile([P_TILE, NXYZ], f32)
            nc.gpsimd.dma_start(inp_b[:], inp_d.ap()[:, wa:inp_w])
            ay_b = inp_a[:, NCOEF * n_chunks : NCOEF * n_chunks + NXYZ]
            az_b = inp_a[:, NCOEF * n_chunks + NXYZ : NCOEF * n_chunks + 2 * NXYZ]
            ax_b = inp_b[:, 0:NXYZ]
            coef_off = 0

            # PSUM accumulators [48, 384] per z-piece, accumulated over chunks
            accs = [
                acc_pool.tile([NXYZ, ZBLK * NXYZ], f32, tag=f"acc{b}", name=f"acc{b}")
                for b in range(NPIECE)
            ]

            # Stage 1: per-chunk 1D Gaussian tables (kept alive for all pieces)
            exs, eyzs = [], []
            for c in range(n_chunks):
                o = coef_off + c * NCOEF
                s_c = inp_a[:, o : o + 1]
                ntx = inp_a[:, o + 1 : o + 2]
                nty = inp_a[:, o + 2 : o + 3]
                ntz = inp_a[:, o + 3 : o + 4]
                lw = inp_a[:, o + 4 : o + 5]

                # SQ[:, axis_block] = (s*coord - s*center)^2 = -bw * d^2
                # yz first so the M outer-product (needs eyz only) starts early
                sq = wpool.tile([P_TILE, 3 * NXYZ], f32, tag="sq", name=f"sq{c}")
                if c == 0:
                    # chunk 0 on the idle-at-start Vector engine (tensor_scalar
                    # runs 2x for fp32) to shorten the ACT prefix
                    u = wpool.tile([P_TILE, 2 * NXYZ], f32, tag="u", name="u0")
                    nc.vector.tensor_scalar(
                        u[:, 0:NXYZ], ay_b, s_c, nty, AluOpType.mult, AluOpType.add
                    )
                    nc.vector.tensor_scalar(
                        u[:, NXYZ : 2 * NXYZ], az_b, s_c, ntz, AluOpType.mult, AluOpType.add
                    )
                    nc.vector.tensor_tensor(
                        sq[:, NXYZ : 3 * NXYZ], u[:], u[:], AluOpType.mult
                    )
                else:
                    for blk, bias, coord in ((1, nty, ay_b), (2, ntz, az_b)):
                        sq_i = nc.scalar.activation(
                            sq[:, blk * NXYZ : (blk + 1) * NXYZ],
                            coord,
                            ACT.Square,
                            bias=bias,
                            scale=s_c,
                        )
                        # keep later-chunk ACT work behind chunk 0's critical
                        # Exp so it can't jump the in-order ACT queue
                        add_dep_helper(
                            sq_i.ins, exp_yz0.ins, sync=False, reason="act order"
                        )
                eyz_dt = mm_dt if MM_DTYPE == "bf16" else f32
                eyz = wpool.tile(
                    [P_TILE, 2 * NXYZ], eyz_dt, tag="eyz", name=f"eyz{c}", bufs=n_chunks
                )
                exp_yz = nc.scalar.activation(
                    eyz[:], sq[:, NXYZ : 3 * NXYZ], ACT.Exp, bias=0.0, scale=-1.0
                )
                if c == 0:
                    exp_yz0 = exp_yz
                # Ex = exp(-SQx + log w)  (weight folded in)
                nc.scalar.activation(
                    sq[:, 0:NXYZ], ax_b, ACT.Square, bias=ntx, scale=s_c
                )
                ex = wpool.tile([P_TILE, NXYZ], mm_dt, tag="ex", name=f"ex{c}", bufs=n_chunks)
                nc.scalar.activation(ex[:], sq[:, 0:NXYZ], ACT.Exp, bias=lw, scale=-1.0)
                exs.append(ex)
                eyzs.append(eyz)

            # Stage 2: c-outer sweeps (no DVE head-of-line stall on chunk-1
            # tables); piece b drains during the final sweep right after its
            # stop matmul, overlapping the remaining pieces' work
            out_s = opool.tile([NXYZ, G2D], f32)
            for c in range(n_chunks):
                for b in range(NPIECE):
                    ey = eyzs[c][:, 0:NXYZ]
                    ez_b = eyzs[c][:, NXYZ + b * ZBLK : NXYZ + (b + 1) * ZBLK]
                    m_t = mpool.tile(
                        [P_TILE, ZBLK * NXYZ], mm_dt, tag="m", name=f"m{b}_{c}", bufs=8
                    )
                    nc.vector.tensor_tensor(
                        m_t[:].rearrange("p (z j) -> p z j", z=ZBLK),
                        ey.unsqueeze(1).broadcast_to((P_TILE, ZBLK, NXYZ)),
                        ez_b.unsqueeze(2).broadcast_to((P_TILE, ZBLK, NXYZ)),
                        AluOpType.mult,
                    )
                    nc.tensor.matmul(
                        accs[b][:],
                        exs[c][:],
                        m_t[:],
                        start=(c == 0),
                        stop=(c == n_chunks - 1),
                    )

                    if c == n_chunks - 1:
                        # drain piece b: psum -> sbuf on Scalar (idle after
                        # tables; keeps Vector on pure outer-product work)
                        dst = out_s[:, b * ZBLK * NXYZ : (b + 1) * ZBLK * NXYZ]
                        nc.scalar.copy(dst, accs[b][:])
                        if b % 2 == 1:
                            # one DMA per piece-pair, rotating queues (i-major;
                            # host transposes to (z,j,i) while summing partials)
                            lo = (b - 1) * ZBLK * NXYZ
                            hi = (b + 1) * ZBLK * NXYZ
                            pair = out_s[:, lo:hi]
                            dens_pc = dens_d.ap().rearrange(
                                "(i zj) -> i zj", i=NXYZ
                            )[:, lo:hi]
                            dma_eng = (nc.sync, nc.gpsimd, nc.scalar)[(b - 1) // 2]
                            with nc.allow_non_contiguous_dma("strided output store"):
                                dma_eng.dma_start(dens_pc, pair)

    nc.compile()
    return nc


def _host_prep(X, aw, bw, elements, C_expand, real_grid):
    ax = np.ascontiguousarray(real_grid[0:NXYZ, 0])
    ay = np.ascontiguousarray(real_grid[0 : NXYZ * NXYZ : NXYZ, 1])
    az = np.ascontiguousarray(real_grid[0 : G : NXYZ * NXYZ, 2])

    mask = (elements != 5) & (C_expand == 1)
    act = np.nonzero(mask)[0]
    # per-(atom, gaussian) flattened arrays over active atoms
    bw_p = bw[act].reshape(-1).astype(np.float64)
    aw_p = aw[act].reshape(-1).astype(np.float64)
    x_p = np.repeat(X[act, 0].astype(np.float64), 6)
    y_p = np.repeat(X[act, 1].astype(np.float64), 6)
    z_p = np.repeat(X[act, 2].astype(np.float64), 6)
    p_act = bw_p.shape[0]

    per_core = max(1, math.ceil(p_act / (N_CORES * P_TILE))) * P_TILE
    n_chunks = per_core // P_TILE

    s_p = np.sqrt(-bw_p)
    coef_full = np.zeros((N_CORES * per_core, NCOEF), dtype=np.float32)
    coef_full[:, 4] = -1e4  # padding rows: exp(-1e4) -> 0
    coef_full[:p_act, 0] = s_p
    coef_full[:p_act, 1] = -s_p * x_p
    coef_full[:p_act, 2] = -s_p * y_p
    coef_full[:p_act, 3] = -s_p * z_p
    coef_full[:p_act, 4] = np.log(aw_p)

    # core/chunk/partition layout: [core][chunk][row(128)] -> [row, chunk*NCOEF+j]
    coefs = []
    for core in range(N_CORES):
        cc = coef_full[core * per_core : (core + 1) * per_core]  # [per_core, NCOEF]
        cc = cc.reshape(n_chunks, P_TILE, NCOEF).transpose(1, 0, 2).reshape(P_TILE, -1)
        coefs.append(np.ascontiguousarray(cc))

    # packed per-core input: [axs (3*48) | coef (NCOEF*n_chunks)]
    inps = []
    for core in range(N_CORES):
        nco = NCOEF * n_chunks
        inp = np.empty((P_TILE, 3 * NXYZ + nco), dtype=np.float32)
        inp[:, 0:nco] = coefs[core]
        inp[:, nco : nco + NXYZ] = ay[None, :]
        inp[:, nco + NXYZ : nco + 2 * NXYZ] = az[None, :]
        inp[:, nco + 2 * NXYZ :] = ax[None, :]
        inps.append(inp)
    return inps, n_chunks


_prog_cache = {}


def kernel(X, aw, bw, elements, C_expand, real_grid, _trace=False):
    from concourse import bass_utils

    inps, n_chunks = _host_prep(X, aw, bw, elements, C_expand, real_grid)

    key = (n_chunks, MM_DTYPE)
    if key not in _prog_cache:
        _prog_cache[key] = _build_program(n_chunks)
    nc = _prog_cache[key]

    in_maps = [{"inp": inps[core]} for core in range(N_CORES)]
    res = bass_utils.run_bass_kernel_spmd(
        nc, in_maps, core_ids=list(range(N_CORES)), trace=_trace
    )
    dens = np.zeros((G2D, NXYZ), dtype=np.float64)
    for core in range(N_CORES):
        dens += res.results[core]["dens"].reshape(NXYZ, G2D).T
    out = np.ascontiguousarray(dens.reshape(-1)).astype(np.float32)
    if _trace:
        return out, res
    return out
